# revision 31
# baseline (speedup 1.0000x reference)
"""Trainium2 Bass kernel for nn_MemoryRel (scatter_memory).

Math (validated numerically): with A = H@Wc[:512], C = H@Wc[527:], G = rel_embs@Wc[512:527],
  mem_bank[n=(i,j)] = lrelu( w_n*(A[i]+C[j]) + sum_r E[r,n]*G[r] + bc ),  w = E.sum(r)
Hops: kv = x@Wk[h]+bk; k=tanh(kv[:512]); v=lrelu(kv[512:]);
  s = mem_bank@k; softmax over ALL arcs (mask is all-true for uniform energy);
  mem = softmax(s)@mem_bank; x = lrelu([v|mem]@Wh[h]+bh).

Sharding: arcs (i-dimension) split 8 ways; per-core 48x384=18432 arcs = 144 tiles
of [128 arcs, 512 d]. mem_bank kept fp16 SBUF-resident.

v2 performance structure:
- All matmul moving operands fp16 (f32 rhs costs 4 cycles/row; fp16 costs 1).
- No per-i gaug copies: A-term is a rank-1 matmul using the packed w row.
- Softmax split into NG groups of tiles with group-local maxes: the PE
  u-matvec for group g runs under the DVE score pass of group g+1; group
  partials [m_g, z_g, u_g] ride one AllGather of [NG, 520] per hop and are
  combined across NCORE*NG rows with the same log-sum-exp merge.
- Wk/Wh fp16 streamed from DRAM through a 13-deep pool so weight DMA
  prefetches under the score pass; diag(w) tiles built on the Pool engine;
  psum->sbuf staging copies on DVE so the Activation engine only runs
  lrelu/tanh/exp.
"""
import numpy as np
import ml_dtypes

import concourse.bass as bass
import concourse.bacc as bacc
import concourse.mybir as mybir
import concourse.tile as tile
from concourse.bass_utils import run_bass_kernel_spmd

dt = mybir.dt
AF = mybir.ActivationFunctionType
ALU = mybir.AluOpType

R, L, D, EREL, IN4, HOPS, NCORE = 45, 384, 512, 15, 1024, 3, 8
IPC = L // NCORE            # 48 head-rows per core
NARC = IPC * L              # 18432 arcs per core
NT = NARC // 128            # 144 tiles of 128 arcs
NTH = NT // 2               # 72 tiles per packed E half
NG = 4                      # softmax groups per hop
GT = NT // NG               # 36 tiles per group
ALPHA = 0.01                # leaky_relu slope
AGW = 520                   # AllGather payload width (32B aligned)

f32, fp16 = dt.float32, dt.float16


def _build_module(reps=1):
    nc = bacc.Bacc("TRN2", target_bir_lowering=False, debug=False,
                   num_devices=NCORE)
    rg = [list(range(NCORE))]

    # ---------------- DRAM I/O ----------------
    d_epack = nc.dram_tensor("e_pack", [111, NARC // 2], fp16, kind="ExternalInput")
    d_wcol = nc.dram_tensor("wcol", [128, NT], f32, kind="ExternalInput")
    d_ht = nc.dram_tensor("ht", [128, 4, L], fp16, kind="ExternalInput")      # H^T packed
    d_hti = nc.dram_tensor("hti", [128, 4, IPC], fp16, kind="ExternalInput")  # per-core H^T cols
    d_wc1 = nc.dram_tensor("wc1", [4, 128, D], fp16, kind="ExternalInput")
    d_wc3 = nc.dram_tensor("wc3", [4, 128, D], fp16, kind="ExternalInput")
    d_wc2 = nc.dram_tensor("wc2", [EREL, D], f32, kind="ExternalInput")
    d_relt = nc.dram_tensor("relt", [EREL, R], f32, kind="ExternalInput")
    d_bc = nc.dram_tensor("bcb", [1, D], fp16, kind="ExternalInput")
    d_wk = nc.dram_tensor("wk", [HOPS, 8, 128, IN4], fp16, kind="ExternalInput")
    d_wh = nc.dram_tensor("wh", [HOPS, 8, 128, IN4], fp16, kind="ExternalInput")
    d_bk = nc.dram_tensor("bk2", [HOPS, 2, 1, D], f32, kind="ExternalInput")
    d_bh = nc.dram_tensor("bh2", [HOPS, 2, 1, D], f32, kind="ExternalInput")
    d_x0t = nc.dram_tensor("x0t", [128, 8], fp16, kind="ExternalInput")
    d_idf = nc.dram_tensor("id128f", [128, 128], f32, kind="ExternalInput")
    d_idb = nc.dram_tensor("id128b", [128, 128], fp16, kind="ExternalInput")
    d_out = nc.dram_tensor("out", [1, IN4], fp16, kind="ExternalOutput")

    with tile.TileContext(nc) as tc:
        with (
            tc.tile_pool(name="const", bufs=1) as pc,
            tc.tile_pool(name="mb", bufs=1) as pmb,
            tc.tile_pool(name="stream", bufs=4) as ps5,
            tc.tile_pool(name="wstream", bufs=13) as pws,
            tc.tile_pool(name="aux", bufs=1) as pa,
            tc.tile_pool(name="rot", bufs=2) as prot,
            tc.tile_pool(name="diagp", bufs=3) as pdg,
            tc.tile_pool(name="psb", bufs=3, space="PSUM") as pb,
            tc.tile_pool(name="psv", bufs=2, space="PSUM") as pv,
            tc.tile_pool(name="pss", bufs=3, space="PSUM") as ps,
            tc.tile_pool(name="dram", bufs=2, space="DRAM") as pd,
        ):
            for rep in range(reps):
                _emit_iteration(nc, rg, pc, pmb, ps5, pws, pa, prot, pdg,
                                pb, pv, ps, pd,
                                d_epack, d_wcol, d_ht, d_hti, d_wc1, d_wc3,
                                d_wc2, d_relt, d_bc, d_wk, d_wh, d_bk, d_bh,
                                d_x0t, d_idf, d_idb, d_out)

    nc.compile()
    return nc


def _emit_iteration(nc, rg, pc, pmb, ps5, pws, pa, prot, pdg, pb, pv, ps, pd,
                    d_epack, d_wcol, d_ht, d_hti, d_wc1, d_wc3, d_wc2, d_relt,
                    d_bc, d_wk, d_wh, d_bk, d_bh, d_x0t, d_idf, d_idb, d_out):
    # ---------------- constants / setup ----------------
    junk = pc.tile([1, 8], f32, tag="junk")

    def touch(ap):
        # absorb a DMA-completion wait into a cheap DVE op so a following
        # instruction carries <=1 sync wait
        nc.vector.tensor_copy(junk[0:1, 0:1], ap[0:1, 0:1])

    E_all = pc.tile([111, NARC // 2], fp16, tag="eall")
    nc.sync.dma_start(E_all[:], d_epack[:])
    EB = [0, 64]                         # per-half partition base
    w_sb = pc.tile([128, NT], f32, tag="wsb")
    nc.sync.dma_start(w_sb[:], d_wcol[:])
    hti_sb = pc.tile([128, 4, IPC], fp16, tag="hti")
    nc.sync.dma_start(hti_sb[:], d_hti[:])
    idf = pc.tile([128, 128], f32, tag="idf")
    nc.sync.dma_start(idf[:], d_idf[:])
    idb = pc.tile([128, 128], fp16, tag="idb")
    nc.sync.dma_start(idb[:], d_idb[:])
    touch(idb)
    x0t_sb = pc.tile([128, 8], fp16, tag="x0t")
    nc.sync.dma_start(x0t_sb[:], d_x0t[:])

    ones_row = pc.tile([1, 128], f32, tag="orow")
    nc.vector.memset(ones_row[:], 1.0)
    ones_h = pc.tile([1, 128], fp16, tag="orowh")
    nc.vector.memset(ones_h[:], 1.0)
    ones_col = pc.tile([128, 1], f32, tag="ocol")
    nc.vector.memset(ones_col[:], 1.0)

    # G_aug [46, 512] fp16 at bases 0/64: rows 0..44 = rel_embs @ Wc2, row 45 = bc
    relt_sb = ps5.tile([EREL, R], f32, tag="stream")
    nc.sync.dma_start(relt_sb[:], d_relt[:])
    wc2_sb = ps5.tile([EREL, D], f32, tag="stream")
    nc.sync.dma_start(wc2_sb[:], d_wc2[:])

    G_sb = pc.tile([R + 1, D], fp16, tag="gsb")
    psum_g = pb.tile([R, D], f32, tag="b")
    nc.tensor.matmul(psum_g[:], relt_sb[:], wc2_sb[:], start=True, stop=True)
    nc.scalar.activation(G_sb[0:R, :], psum_g[:], AF.Copy)
    nc.sync.dma_start(G_sb[R:R + 1, :], d_bc[:])

    # A = H[i0:i0+48] @ Wc1  -> [48, 512] fp16 (rhs rows for the rank-1 A-term)
    A_sb = pc.tile([IPC, D], fp16, tag="asb")
    psum_a = pb.tile([IPC, D], f32, tag="b")
    for c in range(4):
        wc1_c = ps5.tile([128, D], fp16, tag="stream")
        nc.sync.dma_start(wc1_c[:], d_wc1[c])
        nc.tensor.matmul(psum_a[:], hti_sb[:, c, :], wc1_c[:],
                         start=(c == 0), stop=(c == 3))
    nc.scalar.activation(A_sb[:], psum_a[:], AF.Copy)

    # C = H @ Wc3 -> [128, 3, 512] fp16
    C_sb = pc.tile([128, 3, D], fp16, tag="csb")
    psum_c = [pb.tile([128, D], f32, tag="b", name=f"psum_c{jm}")
              for jm in range(3)]
    for c in range(4):
        wc3_c = ps5.tile([128, D], fp16, tag="stream")
        nc.sync.dma_start(wc3_c[:], d_wc3[c])
        ht_c = ps5.tile([128, L], fp16, tag="stream")
        nc.sync.dma_start(ht_c[:], d_ht[:, c, :])
        for jm in range(3):
            nc.tensor.matmul(psum_c[jm][:], ht_c[:, 128 * jm:128 * (jm + 1)],
                             wc3_c[:], start=(c == 0), stop=(c == 3))
    for jm in range(3):
        nc.scalar.activation(C_sb[:, jm, :], psum_c[jm][:], AF.Copy)

    # rotating MM1 rhs buffers [G(45); bc; A[iloc]] at bases 0 and 64;
    # only the A row (46 / 110) is rewritten per iloc (HWDGE DMA)
    NGA = 4
    gaug = [pc.tile([111, D], fp16, tag=f"gaug{i}", name=f"gaug{i}")
            for i in range(NGA)]
    for ga in gaug:
        nc.gpsimd.tensor_copy(ga[0:R + 1, :], G_sb[0:R + 1, :])
        nc.sync.dma_start(ga[64:64 + R + 1, :], G_sb[0:R + 1, :])

    # ---------------- persistent state tiles ----------------
    mb_all = pmb.tile([128, NT, D], fp16, tag="mball")
    s_all = pc.tile([128, NT], f32, tag="sall")
    e_b = pc.tile([128, NT], fp16, tag="eb")
    trash = pc.tile([128, D], fp16, tag="trash")
    ag_all = pc.tile([NCORE * NG, AGW], f32, tag="agall")

    def matvec_1024(xT, wdram, bdram, h):
        """[1,1024] = x @ W[h] + b[h] accumulated in two [1,512] psums."""
        psums = []
        for half in range(2):
            p = pv.tile([1, D], f32, tag="v")
            bt = ps5.tile([1, D], f32, tag="stream")
            nc.sync.dma_start(bt[0:1, :], bdram[h, half])
            nc.tensor.matmul(p[:], ones_row[0:1, 0:1], bt[0:1, :],
                             start=True, stop=False)
            for c in range(8):
                wt_ = pws.tile([128, D], fp16, tag="wstr")
                nc.sync.dma_start(wt_[:], wdram[h, c, :, D * half:D * (half + 1)])
                nc.tensor.matmul(p[:], xT[:, c:c + 1], wt_[:],
                                 start=False, stop=(c == 7))
            psums.append(p)
        return psums

    def transpose_1024(xrow, tag):
        """[1,1024] fp16 -> [128, 8] fp16 via 8 rank-1 matmuls."""
        pxt = ps.tile([128, 8], f32, tag="s")
        for c in range(8):
            nc.tensor.matmul(pxt[:, c:c + 1], xrow[0:1, 128 * c:128 * (c + 1)],
                             ones_h[0:1, 0:1], start=True, stop=True)
        xt = prot.tile([128, 8], fp16, tag=tag, bufs=1)
        nc.vector.tensor_copy(xt[:], pxt[:])
        return xt

    def score_tile(t, k_rep):
        nc.vector.scalar_tensor_tensor(
            trash[:], mb_all[:, t, :], 1.0, k_rep[:],
            ALU.mult, ALU.mult, accum_out=s_all[:, t:t + 1])

    def group_tail(g, agi_d):
        """After scores for group g: local max m_g, e=exp(s-m_g), z_g,
        u_g = e @ mb; DMA [m_g, z_g, u_g] into collective input row g."""
        g0 = g * GT
        mx_p = pa.tile([128, 1], f32, tag="mxp")
        nc.vector.tensor_reduce(mx_p[:], s_all[:, g0:g0 + GT],
                                mybir.AxisListType.X, ALU.max)
        psum_mt = ps.tile([1, 128], f32, tag="s")
        nc.tensor.transpose(psum_mt[:], mx_p[:], idf[:])
        m_loc = pa.tile([1, 1], f32, tag="mloc")
        nc.vector.tensor_reduce(m_loc[:], psum_mt[:], mybir.AxisListType.X,
                                ALU.max)
        neg_m = pa.tile([1, 1], f32, tag="negm")
        nc.vector.tensor_scalar(neg_m[:], m_loc[:], -1.0, None, ALU.mult)
        psum_nm = ps.tile([128, 1], f32, tag="s")
        nc.tensor.matmul(psum_nm[:], ones_row[0:1, :], neg_m[0:1, :],
                         start=True, stop=True)
        negm_rep = pa.tile([128, 1], f32, tag="negmrep")
        nc.vector.tensor_copy(negm_rep[:], psum_nm[:])

        z_p = pa.tile([128, 1], f32, tag="zp")
        nc.scalar.activation(e_b[:, g0:g0 + GT], s_all[:, g0:g0 + GT], AF.Exp,
                             bias=negm_rep[:, 0:1], accum_out=z_p[:])

        psum_u = pv.tile([1, D], f32, tag="v")
        for t in range(g0, g0 + GT):
            nc.tensor.matmul(psum_u[:], e_b[:, t:t + 1], mb_all[:, t, :],
                             start=(t == g0), stop=(t == g0 + GT - 1))
        psum_z = ps.tile([1, 1], f32, tag="s")
        nc.tensor.matmul(psum_z[:], z_p[:], ones_col[:], start=True, stop=True)

        zst = pa.tile([1, 1], f32, tag="zst")
        nc.vector.tensor_copy(zst[:], psum_z[:])
        ust = pa.tile([1, D], f32, tag="ust")
        nc.vector.tensor_copy(ust[:], psum_u[:])
        nc.sync.dma_start(agi_d[g:g + 1, 0:1], m_loc[:])
        nc.sync.dma_start(agi_d[g:g + 1, 1:2], zst[:])
        nc.sync.dma_start(agi_d[g:g + 1, 8:8 + D], ust[:])

    def build_tile(t, k_rep):
        """mem_bank tile t + fused hop-0 score."""
        half, tl = t // NTH, t % NTH
        b = EB[half]
        iloc, jm = t // 3, t % 3
        ga = gaug[iloc % NGA]
        if jm == 0:
            # rewrite the A rows for this iloc (HWDGE; hidden under DVE pace)
            nc.sync.dma_start(ga[R + 1:R + 2, :], A_sb[iloc:iloc + 1, :])
            nc.sync.dma_start(ga[64 + R + 1:64 + R + 2, :],
                              A_sb[iloc:iloc + 1, :])
        pbt = pb.tile([128, D], f32, tag="b")
        # [E; ones; w]^T @ [G; bc; A[iloc]] : 47 rows
        nc.tensor.matmul(pbt[:], E_all[b:b + R + 2, 128 * tl:128 * (tl + 1)],
                         ga[b:b + R + 2, :], start=True, stop=False)
        # diag(w) @ C block (dg on the otherwise-idle Pool engine, keeping
        # DVE free for the fused hop-0 score pass)
        dg = pdg.tile([128, 128], fp16, tag="diag")
        nc.gpsimd.tensor_scalar(dg[:], idb[:], w_sb[:, t:t + 1], None, ALU.mult)
        nc.tensor.matmul(pbt[:], dg[:], C_sb[:, jm, :], start=False, stop=True)
        nc.scalar.activation(mb_all[:, t, :], pbt[:], AF.Lrelu, alpha=ALPHA)
        score_tile(t, k_rep)

    def combine_and_next(h, x_cat, agi_d):
        """AllGather group partials, log-sum-exp merge, finish x_cat, then
        x_next = lrelu(x_cat @ Wh[h] + bh[h])."""
        ago_d = pd.tile([NCORE * NG, AGW], f32, tag="ago")
        nc.gpsimd.collective_compute(
            "AllGather", ALU.bypass, ins=[agi_d.opt()], outs=[ago_d.opt()],
            replica_groups=rg)
        nc.sync.dma_start(ag_all[:], ago_d[:])
        touch(ag_all)

        NR = NCORE * NG
        psum_m8 = ps.tile([1, NR], f32, tag="s")
        nc.tensor.transpose(psum_m8[:], ag_all[:, 0:1], idf[0:NR, 0:NR])
        mg = pa.tile([1, 1], f32, tag="mg")
        nc.vector.tensor_reduce(mg[:], psum_m8[:], mybir.AxisListType.X, ALU.max)
        neg_mg = pa.tile([1, 1], f32, tag="negmg")
        nc.vector.tensor_scalar(neg_mg[:], mg[:], -1.0, None, ALU.mult)
        psum_b8 = ps.tile([NR, 1], f32, tag="s")
        nc.tensor.matmul(psum_b8[:], ones_row[0:1, 0:NR], neg_mg[0:1, :],
                         start=True, stop=True)
        neg_mg8 = pa.tile([NR, 1], f32, tag="negmg8")
        nc.vector.tensor_copy(neg_mg8[:], psum_b8[:])
        scale8 = pa.tile([NR, 1], f32, tag="scale8")
        nc.scalar.activation(scale8[:], ag_all[:, 0:1], AF.Exp,
                             bias=neg_mg8[:, 0:1])
        # u = scale8^T @ u_rows, z = scale8^T @ z_col (PE does the weighting)
        psum_ug = pv.tile([1, D], f32, tag="v")
        nc.tensor.matmul(psum_ug[:], scale8[:, 0:1], ag_all[:, 8:8 + D],
                         start=True, stop=True)
        psum_zg = ps.tile([1, 1], f32, tag="s")
        nc.tensor.matmul(psum_zg[:], scale8[:, 0:1], ag_all[:, 1:2],
                         start=True, stop=True)
        rz = pa.tile([1, 1], f32, tag="rz")
        nc.vector.reciprocal(rz[:], psum_zg[:])
        nc.vector.tensor_scalar(x_cat[0:1, D:IN4], psum_ug[:], rz[0:1, 0:1],
                                None, ALU.mult)

        xcT = transpose_1024(x_cat, "xct")
        wh_a, wh_b = matvec_1024(xcT, d_wh, d_bh, h)
        x_next = prot.tile([1, IN4], fp16, tag="xnext", bufs=1)
        nc.scalar.activation(x_next[0:1, 0:D], wh_a[:], AF.Lrelu, alpha=ALPHA)
        nc.scalar.activation(x_next[0:1, D:IN4], wh_b[:], AF.Lrelu, alpha=ALPHA)
        return x_next

    def hop_head(x_cur_T, h):
        """kv matvec, k/v nonlinearity, k replicated to 128 partitions."""
        kv_a, kv_b = matvec_1024(x_cur_T, d_wk, d_bk, h)
        x_cat = prot.tile([1, IN4], fp16, tag="xcat", bufs=1)
        k_half = pa.tile([1, D], fp16, tag="khalf")
        nc.scalar.activation(k_half[0:1, :], kv_a[:], AF.Tanh)
        nc.scalar.activation(x_cat[0:1, 0:D], kv_b[:], AF.Lrelu, alpha=ALPHA)
        psum_kr = pb.tile([128, D], f32, tag="b")
        nc.tensor.matmul(psum_kr[:], ones_h[0:1, :], k_half[0:1, :],
                         start=True, stop=True)
        k_rep = prot.tile([128, D], fp16, tag="krep", bufs=1)
        nc.vector.tensor_copy(k_rep[:], psum_kr[:])
        return x_cat, k_rep

    # ---------------- hops ----------------
    x_cur_T = x0t_sb      # [128, 8] transposed inputs-vector
    x3 = None
    for h in range(HOPS):
        x_cat, k_rep = hop_head(x_cur_T, h)
        agi_d = pd.tile([NG, AGW], f32, tag="agi")
        if h == 0:
            for t in range(NT):
                build_tile(t, k_rep)
                if t % GT == GT - 1:
                    group_tail(t // GT, agi_d)
        else:
            for t in range(NT):
                score_tile(t, k_rep)
                if t % GT == GT - 1:
                    group_tail(t // GT, agi_d)
        x_next = combine_and_next(h, x_cat, agi_d)
        if h < HOPS - 1:
            x_cur_T = transpose_1024(x_next, "xnt")
        else:
            x3 = x_next

    nc.sync.dma_start(d_out[:], x3[:])


_NC_CACHE = {}


def _get_nc(reps=1):
    if reps not in _NC_CACHE:
        _NC_CACHE[reps] = _build_module(reps)
    return _NC_CACHE[reps]


def _prep_inputs(energy, word_h, e1, e2, rel_embs, Wc, bc, Wk, bk, Wh, bh):
    """Host-side sharding / packing (data movement only)."""
    h16 = ml_dtypes.float16 if hasattr(ml_dtypes, "float16") else np.float16
    energy = np.asarray(energy, np.float32)
    H = np.asarray(word_h, np.float32)[0]                      # [L, D]
    Wc = np.asarray(Wc, np.float32)
    HT = np.ascontiguousarray(H.T)                             # [D, L]
    ht = HT.reshape(4, 128, L).transpose(1, 0, 2).astype(h16)  # [128,4,L]
    wc1 = np.ascontiguousarray(Wc[:D].reshape(4, 128, D)).astype(h16)
    wc3 = np.ascontiguousarray(Wc[D + EREL:].reshape(4, 128, D)).astype(h16)
    wc2 = np.ascontiguousarray(Wc[D:D + EREL])
    relt = np.ascontiguousarray(np.asarray(rel_embs, np.float32).T)
    bcb = np.asarray(bc, np.float32).reshape(1, D).astype(h16)
    wk = np.ascontiguousarray(
        np.asarray(Wk, np.float32).reshape(HOPS, 8, 128, IN4)).astype(h16)
    wh = np.ascontiguousarray(
        np.asarray(Wh, np.float32).reshape(HOPS, 8, 128, IN4)).astype(h16)
    bk2 = np.ascontiguousarray(np.asarray(bk, np.float32).reshape(HOPS, 2, 1, D))
    bh2 = np.ascontiguousarray(np.asarray(bh, np.float32).reshape(HOPS, 2, 1, D))
    x0 = np.concatenate([np.asarray(e1, np.float32), np.asarray(e2, np.float32)])
    x0t = np.ascontiguousarray(x0.reshape(8, 128).T).astype(h16)
    idf = np.eye(128, dtype=np.float32)
    idb = np.eye(128, dtype=h16)

    shared = dict(ht=ht, wc1=wc1, wc3=wc3, wc2=wc2, relt=relt, bcb=bcb,
                  wk=wk, wh=wh, bk2=bk2, bh2=bh2, x0t=x0t, id128f=idf,
                  id128b=idb)

    in_maps = []
    ones_row = np.ones((1, NARC), np.float32)
    for c in range(NCORE):
        E = energy[0][:, c * IPC:(c + 1) * IPC, :].reshape(R, NARC)
        w_row = E.sum(axis=0, keepdims=True)                   # [1, 18432]
        E47 = np.concatenate([E, ones_row, w_row], axis=0)     # [47, 18432]
        e_pack = np.zeros((111, NARC // 2), dtype=h16)
        e_pack[0:47] = E47[:, :NARC // 2].astype(h16)
        e_pack[64:111] = E47[:, NARC // 2:].astype(h16)
        wcol = np.ascontiguousarray(w_row.reshape(NT, 128).T)  # [128, 144]
        hti = ht[:, :, c * IPC:(c + 1) * IPC].copy()
        in_maps.append(dict(e_pack=e_pack, wcol=wcol, hti=hti, **shared))
    return in_maps


def kernel(**inputs):
    in_maps = _prep_inputs(
        inputs["energy"], inputs["word_h"], inputs["e1"], inputs["e2"],
        inputs["rel_embs"], inputs["Wc"], inputs["bc"], inputs["Wk"],
        inputs["bk"], inputs["Wh"], inputs["bh"])
    nc = _get_nc()
    res = run_bass_kernel_spmd(nc, in_maps, list(range(NCORE)))
    return np.asarray(res.results[0]["out"], np.float32).reshape(IN4)
